# revision 24
# baseline (speedup 1.0000x reference)
"""Single-head causal attention on 8 TRN2 NeuronCores, data-parallel over batch.

Per core (one batch element):
  x [T=2048, D=1024] fp32, Wq/Wk/Wv [D, H=64]
  out = softmax_causal((x Wq)(x Wk)^T / sqrt(H)) @ (x Wv)   [T, H]

v2 layout strategy:
  - x^T is pre-transposed + pre-cast to bf16 on the HOST into
    xTc [128(p), NCH, ND, TCH] (d = c*128+p on partitions): halves HBM
    traffic vs fp32 and removes all on-device x transposes.
  - qk^T [2H=128, TCH] via one stationary [Wq|Wk] matmul group per chunk;
    the PSUM result is evacuated twice (partitions 0-63 AND 64-127) into
    qk2 [128, 2, T] so that S^T tiles can be computed as PAIRS of
    concurrent K=64 matmuls on PE row groups 0-1 / 2-3 (tile_position
    derives from the operands' base partitions).
  - Each ST pair lands in one [128, 2*TCH] PSUM tile (2 banks); ONE
    ScalarE exp covers both tiles (halves the per-instruction ACT
    overhead; ACT is the serial bottleneck of phase B).
  - v^T [64, T] via Wv-stationary matmuls, PE-transposed into v tiles
    [128(s), H+1] with a ones column (row H of the PV accumulator then
    holds the softmax denominator for free).
  - out^T accum [H+1, TCH] += v_tile.T @ P per s-tile; PE-transpose back,
    reciprocal-normalize, DMA out per chunk.
  - HAM pre-warm: ~8 junk matmuls gated only on a DVE memset keep the PE
    activity window busy from the end of the framework preamble so real
    work runs at 2.4 GHz, not 1.2.

Dtypes: all matmuls bf16 (softmax accumulation fp32 in PSUM);
rel err ~4e-3 vs the fp32 reference.
"""

import numpy as np

import concourse.bass as bass
import concourse.tile as tile
from concourse import bacc, mybir
from concourse.bass_utils import run_bass_kernel_spmd

F32 = mybir.dt.float32
BF16 = mybir.dt.bfloat16

P = 128  # partitions
TCH = 512  # t-chunk (matmul moving free dim)


def emit_attention(tc, cfg):
    from contextlib import ExitStack

    with ExitStack() as ctx:
        _emit_attention(ctx, tc, cfg)


def _emit_attention(ctx, tc, cfg):
    nc = tc.nc
    T, D, H = cfg["T"], cfg["D"], cfg["H"]
    scale = 1.0 / float(np.sqrt(H))
    ND = D // P  # d-chunks
    NCH = T // TCH  # t-chunks
    NT = T // P  # t-tiles
    JT = TCH // P  # t-tiles per chunk (4)

    xT_d = nc.dram_tensor("xTc", [P, NCH, ND, TCH], BF16, kind="ExternalInput").ap()
    wqk_d = nc.dram_tensor("wqkc", [P, ND, 2 * H], BF16, kind="ExternalInput").ap()
    wv_d = nc.dram_tensor("wvc", [P, ND, H], BF16, kind="ExternalInput").ap()
    idh_d = nc.dram_tensor("identHc", [H + 1, H + 1], BF16, kind="ExternalInput").ap()
    tri_d = nc.dram_tensor("tric", [P, P], BF16, kind="ExternalInput").ap()
    out_d = nc.dram_tensor("out", [T, H], F32, kind="ExternalOutput").ap()

    consts = ctx.enter_context(tc.tile_pool(name="consts", bufs=1))
    sbuf = ctx.enter_context(tc.tile_pool(name="sbuf", bufs=1))
    p_p = ctx.enter_context(tc.tile_pool(name="ptile", bufs=6))
    vt_p = ctx.enter_context(tc.tile_pool(name="vt", bufs=2))
    ot_p = ctx.enter_context(tc.tile_pool(name="otile", bufs=2))
    rcp_p = ctx.enter_context(tc.tile_pool(name="rcp", bufs=4))

    # PSUM: 8 banks total = st 2x2 + qkv 3x1 + o 1x1
    ps_st = ctx.enter_context(tc.tile_pool(name="ps_st", bufs=2, space="PSUM"))
    ps_qkv = ctx.enter_context(tc.tile_pool(name="ps_qkv", bufs=3, space="PSUM"))
    ps_o = ctx.enter_context(tc.tile_pool(name="ps_o", bufs=1, space="PSUM"))

    # --- HAM pre-warm: gated only on a gpsimd memset, so the PE busy-window
    # fills from the end of the framework preamble (real work starts warm).
    # M=1 stationary keeps the per-matmul LDWEIGHTS negligible -- the HAM
    # activity monitor only counts MATMUL-busy, so exposed LDW gaps would
    # keep the duty cycle below the un-throttle threshold.
    junk = consts.tile([P, TCH], BF16)
    nc.gpsimd.memset(junk, 0.0)
    with tc.high_priority():
        warm = ps_o.tile([1, TCH], F32, tag="o")
        NWARM = 8
        for i in range(NWARM):
            # one accumulation group: no inter-matmul semaphores, so the
            # stream stays dense enough for the HAM activity window
            nc.tensor.matmul(
                warm[:], junk[:, 0:1], junk[:],
                start=(i == 0), stop=(i == NWARM - 1),
            )
    # dummy reader: creates the WAR edge that keeps the warm group ordered
    # before po's reuse of this PSUM slot (a reader-less write group has
    # no WAW barrier against the next ring allocation)
    wsink = consts.tile([1, 1], F32)
    nc.vector.tensor_copy(wsink[:], warm[:, 0:1])

    # --- constants + input DMAs (chunk 0 first, in d-pair quarters) -------
    ident_h = consts.tile([H + 1, H + 1], BF16)
    tri = consts.tile([P, P], BF16)
    wqk = consts.tile([P, ND, 2 * H], BF16)
    wv = consts.tile([P, ND, H], BF16)
    nc.scalar.dma_start(wqk[:], wqk_d[:])
    nc.scalar.dma_start(wv[:], wv_d[:])
    nc.scalar.dma_start(tri[:], tri_d[:])
    nc.scalar.dma_start(ident_h[:], idh_d[:])

    xT = sbuf.tile([P, NCH, ND, TCH], BF16)
    # tiny dummy transfer absorbs the DMA rings' cold-start latency before
    # the first real x push lands
    dwarm = consts.tile([P, 32], BF16)
    with tc.high_priority():
        nc.sync.dma_start(dwarm[:], xT_d[:, 0, 0, 0:32])
    for q in range(4):  # chunk 0 in d-pair quarters: push cost is ~700ns each
        nc.sync.dma_start(xT[:, 0, 2 * q : 2 * q + 2, :], xT_d[:, 0, 2 * q : 2 * q + 2, :])
    for c in range(1, NCH):
        nc.sync.dma_start(xT[:, c], xT_d[:, c])

    # --- persistent activations -----------------------------------------
    # ST pair operands must share a base partition per row-group (walrus:
    # "Fmap and Weight must start at the same partition index"), so k^T is
    # kept on BOTH halves (persistent kk2) and q^T is duplicated to the
    # upper half per chunk (transient, only this chunk's phase B reads it).
    kk2 = sbuf.tile([P, T], BF16)
    v_sb = sbuf.tile([P, NT, H + 1], BF16)  # v tiles + ones column
    nc.vector.memset(v_sb[:, :, H : H + 1], 1.0)
    o_sb = sbuf.tile([P, NT, H], F32)  # final normalized output staging

    out_dst = out_d.rearrange("(j p) h -> p j h", p=P)  # [128, NT, H]

    def phase_a_ops(c):
        """Thunk list projecting chunk c (q/k into qk2, v into v_sb)."""
        ops = []
        tsl = slice(c * TCH, (c + 1) * TCH)

        pqk = ps_qkv.tile([2 * H, TCH], F32, tag="qkv")
        for d0 in range(0, ND, 2):
            def qk_mm(d0=d0):
                for d in (d0, d0 + 1):
                    nc.tensor.matmul(
                        pqk[:], wqk[:, d, :], xT[:, c, d, :],
                        start=(d == 0), stop=(d == ND - 1),
                    )
            ops.append(qk_mm)

        # evac the native [q; k] PSUM once (column-bound on DVE, so the
        # full-width cast costs the same as a half), then fan out the two
        # k copies and the q upper-half duplicate as cheap bf16 copies.
        qkf = vt_p.tile([P, 2, TCH], BF16, tag="qkf")
        ops.append(lambda: nc.vector.tensor_copy(qkf[:, 0, :], pqk[:]))

        def qk_dup():
            nc.vector.tensor_copy(kk2[0:H, tsl], qkf[H : 2 * H, 0, :])
            nc.vector.tensor_copy(kk2[H : 2 * H, tsl], qkf[H : 2 * H, 0, :])
            nc.vector.tensor_copy(qkf[H : 2 * H, 1, :], qkf[0:H, 0, :])
        ops.append(qk_dup)

        # v projection as concurrent col-group pairs: even d-chunks write
        # partitions 0-63 of one bank, odd d-chunks partitions 64-127 of
        # another; the vT evac fuses the final add.
        pvA = ps_qkv.tile([H, TCH], F32, tag="qkv")
        pvB = ps_qkv.tile([P, TCH], F32, tag="qkv")
        for dp0 in range(0, ND // 2, 2):
            def v_mm(dp0=dp0):
                for dp in (dp0, dp0 + 1):
                    nc.tensor.matmul(
                        pvA[:], wv[:, 2 * dp, :], xT[:, c, 2 * dp, :],
                        start=(dp == 0), stop=(dp == ND // 2 - 1),
                    )
                    nc.tensor.matmul(
                        pvB[H : 2 * H, :], wv[:, 2 * dp + 1, :],
                        xT[:, c, 2 * dp + 1, :],
                        start=(dp == 0), stop=(dp == ND // 2 - 1),
                    )
            ops.append(v_mm)

        # vT = pvA + pvB; DVE may read only one PSUM operand per
        # instruction, so stage the B half through SBUF first.
        vT = vt_p.tile([H, TCH], BF16, tag="vt")
        vTb = vt_p.tile([H, TCH], BF16, tag="vtb")
        ops.append(lambda: nc.vector.tensor_copy(vTb[:], pvB[H : 2 * H, :]))
        ops.append(lambda: nc.vector.tensor_add(vT[:], pvA[:], vTb[:]))

        def vt_one(j):
            tt = c * JT + j
            pvt = ps_qkv.tile([P, H], BF16, tag="qkv")
            nc.tensor.transpose(
                pvt[:, 0:H], vT[:, j * P : (j + 1) * P], ident_h[0:H, 0:H]
            )
            nc.vector.tensor_copy(v_sb[:, tt, 0:H], pvt[:, 0:H])

        for j in range(JT):
            ops.append(lambda j=j: vt_one(j))
        return ops, qkf

    from collections import deque

    # Global software pipeline: the in-order PE queue stalls on any op
    # whose gate (usually an exp) hasn't fired, so PV pairs are emitted
    # LAG pairs behind their ST/exp (the p_t tiles buffer in SBUF and the
    # exp is long done by the time the PE reaches the PV).
    pending = deque()
    LAG = 2

    def chunk_work(c, qkf, filler):
        """Emit chunk c's ST/exp stream; queue its PVs + epilogue."""
        n_s = (c + 1) * JT
        n_p = n_s // 2
        los = [max(0, (st - c * JT) * P) for st in range(n_s)]
        p_tiles = [None] * n_p
        po = ps_o.tile([H + 1, TCH], F32, tag="o")

        def st_exp_pair(i):
            sA, sB = 2 * i, 2 * i + 1
            loA, loB = los[sA], los[sB]
            pst = ps_st.tile([P, 2 * TCH], F32, tag="st")
            nc.tensor.matmul(
                pst[:, loA:TCH],
                kk2[0:H, sA * P : (sA + 1) * P],
                qkf[0:H, 0, loA:TCH],
                start=True, stop=True,
            )
            nc.tensor.matmul(
                pst[:, TCH + loB : 2 * TCH],
                kk2[H : 2 * H, sB * P : (sB + 1) * P],
                qkf[H : 2 * H, 1, loB:TCH],
                start=True, stop=True,
            )
            p_t = p_p.tile([P, 2 * TCH], BF16, tag="p")
            nc.scalar.activation(
                p_t[:, loA : 2 * TCH], pst[:, loA : 2 * TCH],
                mybir.ActivationFunctionType.Exp, scale=scale,
            )
            if sA >= c * JT:  # diagonal pair: mask the boundary blocks
                nc.vector.tensor_mul(p_t[:, loA : loA + P], p_t[:, loA : loA + P], tri[:])
                nc.vector.tensor_mul(
                    p_t[:, TCH + loB : TCH + loB + P],
                    p_t[:, TCH + loB : TCH + loB + P],
                    tri[:],
                )
            p_tiles[i] = p_t

        def pv_pair(i):
            loA, loB = los[2 * i], los[2 * i + 1]
            p_t = p_tiles[i]
            nc.tensor.matmul(
                po[:, loA:TCH], v_sb[:, 2 * i, :], p_t[:, loA:TCH],
                start=(i == 0), stop=False,
            )
            nc.tensor.matmul(
                po[:, loB:TCH], v_sb[:, 2 * i + 1, :],
                p_t[:, TCH + loB : 2 * TCH],
                start=False, stop=(i == n_p - 1),
            )

        def epilogue():
            # normalize + transpose back to [t, H] + store this chunk
            oT_sb = ot_p.tile([H + 1, TCH], BF16, tag="ot")
            nc.vector.tensor_copy(oT_sb[:], po[:])
            for j in range(JT):
                tt = c * JT + j
                pot = ps_qkv.tile([P, H + 1], BF16, tag="qkv")
                nc.tensor.transpose(
                    pot[:, 0 : H + 1], oT_sb[:, j * P : (j + 1) * P], ident_h[:]
                )
                rcp = rcp_p.tile([P, 1], F32, tag="rcp")
                nc.vector.reciprocal(rcp[:], pot[:, H : H + 1])
                nc.vector.tensor_scalar_mul(o_sb[:, tt, :], pot[:, 0:H], rcp[:])
            nc.sync.dma_start(
                out_dst[:, c * JT : (c + 1) * JT, :],
                o_sb[:, c * JT : (c + 1) * JT, :],
            )

        n_fill = len(filler)
        done_fill = 0
        for i in range(n_p):
            st_exp_pair(i)
            pending.append(lambda i=i: pv_pair(i))
            want = (i + 1) * n_fill // n_p
            while done_fill < want:
                filler[done_fill]()
                done_fill += 1
            while len(pending) > LAG:
                pending.popleft()()
        pending.append(epilogue)

    ops0, qkf_c = phase_a_ops(0)
    for op in ops0:
        op()
    for c in range(NCH):
        if c + 1 < NCH:
            filler, qkf_next = phase_a_ops(c + 1)
        else:
            filler, qkf_next = [], None
        chunk_work(c, qkf_c, filler)
        qkf_c = qkf_next
    while pending:
        pending.popleft()()


def build_nc(cfg):
    nc = bacc.Bacc("TRN2", target_bir_lowering=False, debug=False)
    with tile.TileContext(nc) as tc:
        emit_attention(tc, cfg)
    nc.compile()
    return nc


FULL_CFG = {"T": 2048, "D": 1024, "H": 64}
N_CORES = 8

_nc = None


def host_prep(x, Wq, Wk, Wv, cfg):
    """Pre-transposed bf16 x + stacked weights + identity/causal-mask
    constants, keyed as the kernel's ExternalInputs."""
    import ml_dtypes

    bf = ml_dtypes.bfloat16
    D, H, T = cfg["D"], cfg["H"], cfg["T"]
    ND = D // P
    NCH = T // TCH
    wqk = np.concatenate([Wq, Wk], axis=1).reshape(ND, P, 2 * H).transpose(1, 0, 2)
    wv = Wv.reshape(ND, P, H).transpose(1, 0, 2)
    consts = {
        "wqkc": np.ascontiguousarray(wqk).astype(bf),
        "wvc": np.ascontiguousarray(wv).astype(bf),
        "identHc": np.eye(H + 1, dtype=np.float32).astype(bf),
        "tric": np.triu(np.ones((P, P), dtype=np.float32)).astype(bf),
    }
    # xTc[p, ch, dc, t'] = x[b, ch*TCH+t', dc*P+p]
    xts = []
    for b in range(x.shape[0]):
        xt = x[b].reshape(NCH, TCH, ND, P).transpose(3, 0, 2, 1)
        xts.append(np.ascontiguousarray(xt).astype(bf))
    return xts, consts


def kernel(x, Wq, Wk, Wv, trace=False):
    global _nc
    if _nc is None:
        _nc = build_nc(FULL_CFG)
    Wq = np.ascontiguousarray(Wq, dtype=np.float32)
    Wk = np.ascontiguousarray(Wk, dtype=np.float32)
    Wv = np.ascontiguousarray(Wv, dtype=np.float32)
    x = np.ascontiguousarray(x, dtype=np.float32)
    xts, consts = host_prep(x, Wq, Wk, Wv, FULL_CFG)
    in_maps = [{"xTc": xts[b], **consts} for b in range(N_CORES)]
    res = run_bass_kernel_spmd(_nc, in_maps, core_ids=list(range(N_CORES)), trace=trace)
    out = np.stack([res.results[b]["out"] for b in range(N_CORES)])
    if trace:
        return out, res
    return out


# revision 25
# speedup vs baseline: 1.1940x; 1.1940x over previous
"""Single-head causal attention on 8 TRN2 NeuronCores, data-parallel over batch.

Per core (one batch element):
  x [T=2048, D=1024] fp32, Wq/Wk/Wv [D, H=64]
  out = softmax_causal((x Wq)(x Wk)^T / sqrt(H)) @ (x Wv)   [T, H]

v2 layout strategy:
  - x^T is pre-transposed + pre-cast to bf16 on the HOST into
    xTc [128(p), NCH, ND, TCH] (d = c*128+p on partitions): halves HBM
    traffic vs fp32 and removes all on-device x transposes.
  - qk^T [2H=128, TCH] via one stationary [Wq|Wk] matmul group per chunk;
    the PSUM result is evacuated twice (partitions 0-63 AND 64-127) into
    qk2 [128, 2, T] so that S^T tiles can be computed as PAIRS of
    concurrent K=64 matmuls on PE row groups 0-1 / 2-3 (tile_position
    derives from the operands' base partitions).
  - Each ST pair lands in one [128, 2*TCH] PSUM tile (2 banks); ONE
    ScalarE exp covers both tiles (halves the per-instruction ACT
    overhead; ACT is the serial bottleneck of phase B).
  - v^T [64, T] via Wv-stationary matmuls, PE-transposed into v tiles
    [128(s), H+1] with a ones column (row H of the PV accumulator then
    holds the softmax denominator for free).
  - out^T accum [H+1, TCH] += v_tile.T @ P per s-tile; PE-transpose back,
    reciprocal-normalize, DMA out per chunk.
  - HAM pre-warm: ~8 junk matmuls gated only on a DVE memset keep the PE
    activity window busy from the end of the framework preamble so real
    work runs at 2.4 GHz, not 1.2.

Dtypes: all matmuls bf16 (softmax accumulation fp32 in PSUM);
rel err ~4e-3 vs the fp32 reference.
"""

import numpy as np

import concourse.bass as bass
import concourse.tile as tile
from concourse import bacc, mybir
from concourse.bass_utils import run_bass_kernel_spmd

F32 = mybir.dt.float32
BF16 = mybir.dt.bfloat16

P = 128  # partitions
TCH = 512  # t-chunk (matmul moving free dim)


def emit_attention(tc, cfg):
    from contextlib import ExitStack

    with ExitStack() as ctx:
        _emit_attention(ctx, tc, cfg)


def _emit_attention(ctx, tc, cfg):
    nc = tc.nc
    T, D, H = cfg["T"], cfg["D"], cfg["H"]
    scale = 1.0 / float(np.sqrt(H))
    ND = D // P  # d-chunks
    NCH = T // TCH  # t-chunks
    NT = T // P  # t-tiles
    JT = TCH // P  # t-tiles per chunk (4)

    xT_d = nc.dram_tensor("xTc", [P, NCH, ND, TCH], BF16, kind="ExternalInput").ap()
    wqk_d = nc.dram_tensor("wqkc", [P, ND, 2 * H], BF16, kind="ExternalInput").ap()
    wv_d = nc.dram_tensor("wvc", [P, ND, H], BF16, kind="ExternalInput").ap()
    idh_d = nc.dram_tensor("identHc", [H + 1, H + 1], BF16, kind="ExternalInput").ap()
    tri_d = nc.dram_tensor("tric", [P, P], BF16, kind="ExternalInput").ap()
    out_d = nc.dram_tensor("out", [T, H], F32, kind="ExternalOutput").ap()

    consts = ctx.enter_context(tc.tile_pool(name="consts", bufs=1))
    sbuf = ctx.enter_context(tc.tile_pool(name="sbuf", bufs=1))
    p_p = ctx.enter_context(tc.tile_pool(name="ptile", bufs=6))
    vt_p = ctx.enter_context(tc.tile_pool(name="vt", bufs=2))
    ot_p = ctx.enter_context(tc.tile_pool(name="otile", bufs=2))
    rcp_p = ctx.enter_context(tc.tile_pool(name="rcp", bufs=4))

    # PSUM: 8 banks total = st 2x2 + qkv 3x1 + o 1x1
    ps_st = ctx.enter_context(tc.tile_pool(name="ps_st", bufs=2, space="PSUM"))
    ps_qkv = ctx.enter_context(tc.tile_pool(name="ps_qkv", bufs=3, space="PSUM"))
    ps_o = ctx.enter_context(tc.tile_pool(name="ps_o", bufs=1, space="PSUM"))

    # --- HAM pre-warm: gated only on a gpsimd memset, so the PE busy-window
    # fills from the end of the framework preamble (real work starts warm).
    # M=1 stationary keeps the per-matmul LDWEIGHTS negligible -- the HAM
    # activity monitor only counts MATMUL-busy, so exposed LDW gaps would
    # keep the duty cycle below the un-throttle threshold.
    junk = consts.tile([P, TCH], BF16)
    nc.gpsimd.memset(junk, 0.0)
    with tc.high_priority():
        warm = ps_o.tile([1, TCH], F32, tag="o")
        NWARM = 8
        for i in range(NWARM):
            # one accumulation group: no inter-matmul semaphores, so the
            # stream stays dense enough for the HAM activity window
            nc.tensor.matmul(
                warm[:], junk[:, 0:1], junk[:],
                start=(i == 0), stop=(i == NWARM - 1),
            )
    # dummy reader: creates the WAR edge that keeps the warm group ordered
    # before po's reuse of this PSUM slot (a reader-less write group has
    # no WAW barrier against the next ring allocation)
    wsink = consts.tile([1, 1], F32)
    nc.vector.tensor_copy(wsink[:], warm[:, 0:1])

    # --- constants + input DMAs (chunk 0 first, in d-pair quarters) -------
    ident_h = consts.tile([H + 1, H + 1], BF16)
    tri = consts.tile([P, P], BF16)
    wqk = consts.tile([P, ND, 2 * H], BF16)
    wv = consts.tile([P, ND, H], BF16)
    nc.scalar.dma_start(wqk[:], wqk_d[:])
    nc.scalar.dma_start(wv[:], wv_d[:])
    nc.scalar.dma_start(tri[:], tri_d[:])
    nc.scalar.dma_start(ident_h[:], idh_d[:])

    xT = sbuf.tile([P, NCH, ND, TCH], BF16)
    for d in range(ND):  # chunk 0 per-d so the first projections start early
        nc.sync.dma_start(xT[:, 0, d : d + 1, :], xT_d[:, 0, d : d + 1, :])
    for c in range(1, NCH):
        nc.sync.dma_start(xT[:, c], xT_d[:, c])

    # --- persistent activations -----------------------------------------
    # ST pair operands must share a base partition per row-group (walrus:
    # "Fmap and Weight must start at the same partition index"), so k^T is
    # kept on BOTH halves (persistent kk2) and q^T is duplicated to the
    # upper half per chunk (transient, only this chunk's phase B reads it).
    kk2 = sbuf.tile([P, T], BF16)
    v_sb = sbuf.tile([P, NT, H + 1], BF16)  # v tiles + ones column
    nc.vector.memset(v_sb[:, :, H : H + 1], 1.0)
    o_sb = sbuf.tile([P, NT, H], F32)  # final normalized output staging

    out_dst = out_d.rearrange("(j p) h -> p j h", p=P)  # [128, NT, H]

    def phase_a_ops(c):
        """Thunk list projecting chunk c (q/k into qk2, v into v_sb)."""
        ops = []
        tsl = slice(c * TCH, (c + 1) * TCH)

        pqk = ps_qkv.tile([2 * H, TCH], F32, tag="qkv")
        for d0 in range(0, ND, 2):
            def qk_mm(d0=d0):
                for d in (d0, d0 + 1):
                    nc.tensor.matmul(
                        pqk[:], wqk[:, d, :], xT[:, c, d, :],
                        start=(d == 0), stop=(d == ND - 1),
                    )
            ops.append(qk_mm)

        # evac the native [q; k] PSUM once (column-bound on DVE, so the
        # full-width cast costs the same as a half), then fan out the two
        # k copies and the q upper-half duplicate as cheap bf16 copies.
        qkf = vt_p.tile([P, 2, TCH], BF16, tag="qkf")
        ops.append(lambda: nc.vector.tensor_copy(qkf[:, 0, :], pqk[:]))

        def qk_dup():
            nc.vector.tensor_copy(kk2[0:H, tsl], qkf[H : 2 * H, 0, :])
            nc.vector.tensor_copy(kk2[H : 2 * H, tsl], qkf[H : 2 * H, 0, :])
            nc.vector.tensor_copy(qkf[H : 2 * H, 1, :], qkf[0:H, 0, :])
        ops.append(qk_dup)

        # v projection as concurrent col-group pairs: even d-chunks write
        # partitions 0-63 of one bank, odd d-chunks partitions 64-127 of
        # another; the vT evac fuses the final add.
        pvA = ps_qkv.tile([H, TCH], F32, tag="qkv")
        pvB = ps_qkv.tile([P, TCH], F32, tag="qkv")
        for dp0 in range(0, ND // 2, 2):
            def v_mm(dp0=dp0):
                for dp in (dp0, dp0 + 1):
                    nc.tensor.matmul(
                        pvA[:], wv[:, 2 * dp, :], xT[:, c, 2 * dp, :],
                        start=(dp == 0), stop=(dp == ND // 2 - 1),
                    )
                    nc.tensor.matmul(
                        pvB[H : 2 * H, :], wv[:, 2 * dp + 1, :],
                        xT[:, c, 2 * dp + 1, :],
                        start=(dp == 0), stop=(dp == ND // 2 - 1),
                    )
            ops.append(v_mm)

        # vT = pvA + pvB; DVE may read only one PSUM operand per
        # instruction, so stage the B half through SBUF first.
        vT = vt_p.tile([H, TCH], BF16, tag="vt")
        vTb = vt_p.tile([H, TCH], BF16, tag="vtb")
        ops.append(lambda: nc.vector.tensor_copy(vTb[:], pvB[H : 2 * H, :]))
        ops.append(lambda: nc.vector.tensor_add(vT[:], pvA[:], vTb[:]))

        def vt_one(j):
            tt = c * JT + j
            pvt = ps_qkv.tile([P, H], BF16, tag="qkv")
            nc.tensor.transpose(
                pvt[:, 0:H], vT[:, j * P : (j + 1) * P], ident_h[0:H, 0:H]
            )
            nc.vector.tensor_copy(v_sb[:, tt, 0:H], pvt[:, 0:H])

        for j in range(JT):
            ops.append(lambda j=j: vt_one(j))
        return ops, qkf

    from collections import deque

    # Global software pipeline: the in-order PE queue stalls on any op
    # whose gate (usually an exp) hasn't fired, so PV pairs are emitted
    # LAG pairs behind their ST/exp (the p_t tiles buffer in SBUF and the
    # exp is long done by the time the PE reaches the PV).
    pending = deque()
    LAG = 2

    def chunk_work(c, qkf, filler):
        """Emit chunk c's ST/exp stream; queue its PVs + epilogue."""
        n_s = (c + 1) * JT
        n_p = n_s // 2
        los = [max(0, (st - c * JT) * P) for st in range(n_s)]
        p_tiles = [None] * n_p
        po = ps_o.tile([H + 1, TCH], F32, tag="o")

        def st_exp_pair(i):
            sA, sB = 2 * i, 2 * i + 1
            loA, loB = los[sA], los[sB]
            pst = ps_st.tile([P, 2 * TCH], F32, tag="st")
            nc.tensor.matmul(
                pst[:, loA:TCH],
                kk2[0:H, sA * P : (sA + 1) * P],
                qkf[0:H, 0, loA:TCH],
                start=True, stop=True,
            )
            nc.tensor.matmul(
                pst[:, TCH + loB : 2 * TCH],
                kk2[H : 2 * H, sB * P : (sB + 1) * P],
                qkf[H : 2 * H, 1, loB:TCH],
                start=True, stop=True,
            )
            p_t = p_p.tile([P, 2 * TCH], BF16, tag="p")
            nc.scalar.activation(
                p_t[:, loA : 2 * TCH], pst[:, loA : 2 * TCH],
                mybir.ActivationFunctionType.Exp, scale=scale,
            )
            if sA >= c * JT:  # diagonal pair: mask the boundary blocks
                nc.vector.tensor_mul(p_t[:, loA : loA + P], p_t[:, loA : loA + P], tri[:])
                nc.vector.tensor_mul(
                    p_t[:, TCH + loB : TCH + loB + P],
                    p_t[:, TCH + loB : TCH + loB + P],
                    tri[:],
                )
            p_tiles[i] = p_t

        def pv_pair(i):
            loA, loB = los[2 * i], los[2 * i + 1]
            p_t = p_tiles[i]
            nc.tensor.matmul(
                po[:, loA:TCH], v_sb[:, 2 * i, :], p_t[:, loA:TCH],
                start=(i == 0), stop=False,
            )
            nc.tensor.matmul(
                po[:, loB:TCH], v_sb[:, 2 * i + 1, :],
                p_t[:, TCH + loB : 2 * TCH],
                start=False, stop=(i == n_p - 1),
            )

        def epilogue():
            # normalize + transpose back to [t, H] + store this chunk
            oT_sb = ot_p.tile([H + 1, TCH], BF16, tag="ot")
            nc.vector.tensor_copy(oT_sb[:], po[:])
            for j in range(JT):
                tt = c * JT + j
                pot = ps_qkv.tile([P, H + 1], BF16, tag="qkv")
                nc.tensor.transpose(
                    pot[:, 0 : H + 1], oT_sb[:, j * P : (j + 1) * P], ident_h[:]
                )
                rcp = rcp_p.tile([P, 1], F32, tag="rcp")
                nc.vector.reciprocal(rcp[:], pot[:, H : H + 1])
                nc.vector.tensor_scalar_mul(o_sb[:, tt, :], pot[:, 0:H], rcp[:])
            nc.sync.dma_start(
                out_dst[:, c * JT : (c + 1) * JT, :],
                o_sb[:, c * JT : (c + 1) * JT, :],
            )

        n_fill = len(filler)
        done_fill = 0
        for i in range(n_p):
            st_exp_pair(i)
            pending.append(lambda i=i: pv_pair(i))
            want = (i + 1) * n_fill // n_p
            while done_fill < want:
                filler[done_fill]()
                done_fill += 1
            while len(pending) > LAG:
                pending.popleft()()
        pending.append(epilogue)

    ops0, qkf_c = phase_a_ops(0)
    for op in ops0:
        op()
    for c in range(NCH):
        if c + 1 < NCH:
            filler, qkf_next = phase_a_ops(c + 1)
        else:
            filler, qkf_next = [], None
        chunk_work(c, qkf_c, filler)
        qkf_c = qkf_next
    while pending:
        pending.popleft()()


def build_nc(cfg):
    nc = bacc.Bacc("TRN2", target_bir_lowering=False, debug=False)
    with tile.TileContext(nc) as tc:
        emit_attention(tc, cfg)
    nc.compile()
    return nc


FULL_CFG = {"T": 2048, "D": 1024, "H": 64}
N_CORES = 8

_nc = None


def host_prep(x, Wq, Wk, Wv, cfg):
    """Pre-transposed bf16 x + stacked weights + identity/causal-mask
    constants, keyed as the kernel's ExternalInputs."""
    import ml_dtypes

    bf = ml_dtypes.bfloat16
    D, H, T = cfg["D"], cfg["H"], cfg["T"]
    ND = D // P
    NCH = T // TCH
    wqk = np.concatenate([Wq, Wk], axis=1).reshape(ND, P, 2 * H).transpose(1, 0, 2)
    wv = Wv.reshape(ND, P, H).transpose(1, 0, 2)
    consts = {
        "wqkc": np.ascontiguousarray(wqk).astype(bf),
        "wvc": np.ascontiguousarray(wv).astype(bf),
        "identHc": np.eye(H + 1, dtype=np.float32).astype(bf),
        "tric": np.triu(np.ones((P, P), dtype=np.float32)).astype(bf),
    }
    # xTc[p, ch, dc, t'] = x[b, ch*TCH+t', dc*P+p]
    xts = []
    for b in range(x.shape[0]):
        xt = x[b].reshape(NCH, TCH, ND, P).transpose(3, 0, 2, 1)
        xts.append(np.ascontiguousarray(xt).astype(bf))
    return xts, consts


def kernel(x, Wq, Wk, Wv, trace=False):
    global _nc
    if _nc is None:
        _nc = build_nc(FULL_CFG)
    Wq = np.ascontiguousarray(Wq, dtype=np.float32)
    Wk = np.ascontiguousarray(Wk, dtype=np.float32)
    Wv = np.ascontiguousarray(Wv, dtype=np.float32)
    x = np.ascontiguousarray(x, dtype=np.float32)
    xts, consts = host_prep(x, Wq, Wk, Wv, FULL_CFG)
    in_maps = [{"xTc": xts[b], **consts} for b in range(N_CORES)]
    res = run_bass_kernel_spmd(_nc, in_maps, core_ids=list(range(N_CORES)), trace=trace)
    out = np.stack([res.results[b]["out"] for b in range(N_CORES)])
    if trace:
        return out, res
    return out
